# revision 14
# baseline (speedup 1.0000x reference)
"""Malvar demosaic on 8 trn2 NeuronCores.

Input CFA [16,1,1024,1024] f32 + four 5x5 kernels -> output [16,3,1024,1024].

Strategy (pure data parallel, 2 images per core):
  - Each image is processed in 9 horizontal bands of 124 output rows.
  - The input band is loaded parity-split: the 62 even output rows at
    partitions [0:62], odd at [64:126], the four +-2 halo rows at
    partitions {62,63,126,127} (zero-filled from a zeros input at image
    edges).  Compute-op partition slices therefore always start at 0 or
    64 (a hard TRN2 engine-AP constraint), and one lhsT matrix set works
    for every band.
  - Horizontal +-1/+-2 tap sums S1/S2 are computed on the vector engine
    (free-dim shifted adds); vertical 5-tap mixing (incl. the parity
    permutation) is folded into banded 128x126 matrices applied on the
    tensor engine as float32r matmuls accumulating in PSUM (3 matmuls per
    conv per 512-col half).
  - The Bayer selection is stride-2 quarter-copies from PSUM / the X tile
    into R,G,B planes (split across vector/scalar/gpsimd engines), stored
    with row-parity-strided DMA.
"""

import numpy as np

import concourse.bass as bass
import concourse.mybir as mybir
import concourse.tile as tile
from concourse.bass_utils import run_bass_kernel_spmd

B, H, W = 16, 1024, 1024
N_CORES = 8
IMGS_PER_CORE = B // N_CORES
BAND = 124              # output rows per band
NBANDS = (H + BAND - 1) // BAND   # 9
HALF = 512              # columns per PSUM half
M = 126                 # matmul output partitions (evens [0:62], odds [64:126])
MM_DT = mybir.dt.float32r


def _p_of_d(d):
    """Partition of input row r0+d within the X tile."""
    if 0 <= d < BAND:
        return d // 2 if d % 2 == 0 else 64 + (d - 1) // 2
    return {-2: 62, BAND: 63, -1: 126, BAND + 1: 127}[d]


def _build_matrices(k5s):
    """Packed lhsT [128, 12*M]: per conv, planes X/S1/S2 (band-independent)."""
    packed = np.zeros((128, 12 * M), dtype=np.float32)
    for ki, k5 in enumerate(k5s):
        assert np.allclose(k5[:, 1], k5[:, 3]) and np.allclose(k5[:, 0], k5[:, 4])
        Ms = [np.zeros((128, M), dtype=np.float32) for _ in range(3)]
        for d in range(BAND):                       # output row r0+d
            m = d // 2 if d % 2 == 0 else 64 + (d - 1) // 2
            for dy in range(-2, 3):
                p = _p_of_d(d + dy)
                Ms[0][p, m] += k5[2 + dy, 2]
                Ms[1][p, m] += k5[2 + dy, 1]
                Ms[2][p, m] += k5[2 + dy, 0]
        for pl in range(3):
            packed[:, (ki * 3 + pl) * M:(ki * 3 + pl + 1) * M] = Ms[pl]
    return packed


_CACHE = {}


def _split_waits(nc, max_waits=1):
    """The walrus in this container rejects instructions carrying more than
    one sem wait.  Hoist extra waits onto same-engine NoOps inserted right
    before the offending instruction (sequencer waits are executed in
    program order, so this is semantics-preserving)."""
    total = 0
    for bb in nc.main_func.blocks:
        insts = bb.bb.instructions if hasattr(bb, "bb") else bb.instructions
        i = 0
        while i < len(insts):
            ins = insts[i]
            si = ins.sync_info
            if si is not None and si.on_wait and len(si.on_wait) > max_waits:
                waits = list(si.on_wait)
                keep, hoist = waits[-max_waits:], waits[:-max_waits]
                nops = []
                for w in hoist:
                    nop = mybir.InstNoOp(
                        name=nc.get_next_instruction_name(),
                        engine=ins.engine, ins=[], outs=[],
                        sync_info=mybir.SyncInfo(on_wait=[w], on_update=[]))
                    nc.register_instruction(nop)
                    nops.append(nop)
                ins.sync_info = mybir.SyncInfo(
                    on_wait=keep, on_update=list(si.on_update or []))
                insts[i:i] = nops
                i += len(nops)
                total += len(nops)
            i += 1
    return total


def _build_nc():
    nc = bass.Bass(target_bir_lowering=False, trn_type="TRN2")
    x = nc.dram_tensor("x", [IMGS_PER_CORE, 1, H, W], MM_DT,
                       kind="ExternalInput")
    wts = nc.dram_tensor("wm", [128, 12 * M], MM_DT, kind="ExternalInput")
    zpad = nc.dram_tensor("zpad", [2, W], MM_DT, kind="ExternalInput")
    out = nc.dram_tensor("out", [IMGS_PER_CORE, 3, H, W], mybir.dt.float32,
                         kind="ExternalOutput")

    with tile.TileContext(nc) as tc:
        with (
            tc.tile_pool(name="wpool", bufs=1) as wpool,
            tc.tile_pool(name="xpool", bufs=3) as xpool,
            tc.tile_pool(name="spool", bufs=3) as spool,
            tc.tile_pool(name="opool", bufs=3) as opool,
            tc.tile_pool(name="psum", bufs=2, space="PSUM") as pspool,
        ):
            wt = wpool.tile([128, 12 * M], MM_DT)
            nc.sync.dma_start(wt[:], wts[:])

            for b in range(IMGS_PER_CORE):
                for t in range(NBANDS):
                    r0 = t * BAND
                    n_rows = min(BAND, H - r0)       # stored rows this band
                    n_me = (n_rows + 1) // 2
                    n_mo = n_rows // 2

                    xt = xpool.tile([128, W + 4], MM_DT, tag="x")
                    # main rows [r0, r0+n_rows): evens -> [0:62], odds -> [64:126]
                    nc.sync.dma_start(xt[0:n_me, 2:W + 2],
                                      x[b, 0, r0:r0 + n_rows:2, :])
                    nc.sync.dma_start(xt[64:64 + n_mo, 2:W + 2],
                                      x[b, 0, r0 + 1:r0 + n_rows:2, :])
                    # halo strays: rows {r0-2, r0+124} -> p{62,63},
                    #              rows {r0-1, r0+125} -> p{126,127}
                    for p0, ra, rb in ((62, r0 - 2, r0 + BAND),
                                       (126, r0 - 1, r0 + BAND + 1)):
                        if 0 <= ra and rb < H:
                            nc.sync.dma_start(
                                xt[p0:p0 + 2, 2:W + 2],
                                x[b, 0, ra:rb + 1:rb - ra, :])
                        else:
                            for i, r in enumerate((ra, rb)):
                                nc.sync.dma_start(
                                    xt[p0 + i:p0 + i + 1, 2:W + 2],
                                    x[b, 0, r:r + 1, :] if 0 <= r < H
                                    else zpad[0:1, :])
                    if n_rows < BAND:
                        # first row past the image edge in each parity block
                        # (tapped by the last valid outputs) must be zero
                        nc.sync.dma_start(xt[n_me:n_me + 1, 2:W + 2],
                                          zpad[0:1, :])
                        nc.sync.dma_start(xt[64 + n_mo:64 + n_mo + 1, 2:W + 2],
                                          zpad[0:1, :])
                    nc.gpsimd.memset(xt[:, 0:2].bitcast(mybir.dt.float32), 0.0)
                    nc.gpsimd.memset(
                        xt[:, W + 2:W + 4].bitcast(mybir.dt.float32), 0.0)

                    s1 = spool.tile([128, W], MM_DT, tag="s1")
                    s2 = spool.tile([128, W], MM_DT, tag="s2")
                    nc.vector.tensor_tensor(
                        s1[:], xt[:, 1:W + 1], xt[:, 3:W + 3], mybir.AluOpType.add)
                    nc.vector.tensor_tensor(
                        s2[:], xt[:, 0:W], xt[:, 4:W + 4], mybir.AluOpType.add)

                    # [128, W] so the (p2 m) store view splits as 2 x 64,
                    # matching evens at [0:62] / odds at [64:126]
                    planes = [opool.tile([128, W], mybir.dt.float32, tag=f"pl{i}",
                                         name=f"pl{i}")
                              for i in range(3)]  # R, G, B

                    ev = slice(0, n_me)
                    od = slice(64, 64 + n_mo)
                    for h in range(2):
                        c0 = h * HALF
                        ps = [pspool.tile([M, HALF], mybir.dt.float32,
                                          tag=f"ps{k}", name=f"ps{k}")
                              for k in range(4)]
                        for k in range(4):
                            for pl, rhs in enumerate((
                                    xt[:, c0 + 2:c0 + HALF + 2],
                                    s1[:, c0:c0 + HALF],
                                    s2[:, c0:c0 + HALF])):
                                nc.tensor.matmul(
                                    ps[k][:],
                                    wt[:, (k * 3 + pl) * M:(k * 3 + pl + 1) * M],
                                    rhs,
                                    start=(pl == 0), stop=(pl == 2))

                        # (plane, rows-slice, col-parity, psum index)
                        quarters = [
                            (0, ev, 1, 1), (0, od, 0, 2), (0, od, 1, 3),
                            (1, ev, 0, 0), (1, od, 1, 0),
                            (2, ev, 0, 3), (2, ev, 1, 2), (2, od, 0, 1),
                        ]
                        for qi, (pli, rs, cp, k) in enumerate(quarters):
                            dst = planes[pli][rs, c0 + cp:c0 + HALF:2]
                            srcq = ps[k][rs, cp:HALF:2]
                            if qi % 2 == 0:
                                nc.vector.tensor_copy(dst, srcq)
                            else:
                                nc.scalar.copy(dst, srcq)

                    # X pass-throughs (full width, stride-2 cols)
                    xq = [
                        (0, ev, 2),   # R(e,e) = X
                        (1, ev, 3),   # G(e,o) = X
                        (1, od, 2),   # G(o,e) = X
                        (2, od, 3),   # B(o,o) = X
                    ]
                    for pli, rs, cstart in xq:
                        nc.gpsimd.tensor_copy(
                            planes[pli][rs, cstart - 2:W:2],
                            xt[rs, cstart:cstart + W:2])

                    # stores (row-parity strided)
                    for ch in range(3):
                        nc.sync.dma_start(out[b, ch, r0:r0 + n_rows:2, :],
                                          planes[ch][0:n_me, :])
                        nc.sync.dma_start(out[b, ch, r0 + 1:r0 + n_rows:2, :],
                                          planes[ch][64:64 + n_mo, :])

    _split_waits(nc)
    nc.finalize()
    return nc


def _get_nc():
    if "nc" not in _CACHE:
        _CACHE["nc"] = _build_nc()
    return _CACHE["nc"]


def kernel(CFA_inputs, GR_GB, Rg_RB_Bg_BR, Rg_BR_Bg_RB, Rb_BB_Br_RR, _trace=False):
    cfa = np.ascontiguousarray(np.asarray(CFA_inputs, dtype=np.float32))
    k5s = [np.asarray(k, dtype=np.float32)
           for k in (GR_GB, Rg_RB_Bg_BR, Rg_BR_Bg_RB, Rb_BB_Br_RR)]
    nc = _get_nc()

    wm = _build_matrices(k5s)
    zpad = np.zeros((2, W), dtype=np.float32)
    in_maps = [{"x": cfa[c * IMGS_PER_CORE:(c + 1) * IMGS_PER_CORE],
                "wm": wm, "zpad": zpad} for c in range(N_CORES)]

    res = run_bass_kernel_spmd(nc, in_maps, core_ids=list(range(N_CORES)),
                               trace=_trace)
    outs = np.concatenate([res.results[c]["out"] for c in range(N_CORES)], axis=0)
    if _trace:
        kernel._last = res
    return outs


# revision 16
# speedup vs baseline: 1.3281x; 1.3281x over previous
"""Malvar demosaic on 8 trn2 NeuronCores.

Input CFA [16,1,1024,1024] f32 + four 5x5 kernels -> output [16,3,1024,1024].

Strategy (pure data parallel, 2 images per core):
  - Each image is processed in 9 horizontal bands of 124 output rows.
  - The input band (incl. the +-2 row halo) is loaded parity-split in two
    row-strided DMAs: even rows r0-2..r0+124 at partitions [0:64], odd
    rows r0-1..r0+125 at [64:128].  Out-of-image halo rows are zero-filled
    from a tiny zeros input.
  - The entire Bayer demosaic (four 5x5 convs + per-pixel selection incl.
    the CFA pass-through) is folded into banded 128x126 float32r matmuls:
    for each (output channel, column parity) pair a composite lhsT applies
    the right conv's vertical taps (or the identity) per output row
    parity; the horizontal taps ride on stride-2 rhs slices of X and the
    +-1/+-2 horizontal tap sums S1/S2 (two vector-engine shifted adds).
    3 matmuls per (channel, col-parity) accumulate in one PSUM bank.
  - Each PSUM plane is evicted with a single dense copy into a packed
    [128, 3*1024] output tile (channels side by side), then two
    channel-merged row-parity-strided DMAs store each band.

Every compute-op partition slice starts at 0 or 64 (hard TRN2 engine-AP
constraint) and one lhsT matrix set works for every band (image-edge
zero-padding comes from the zero-filled halo partitions).
"""

import numpy as np

import concourse.bass as bass
import concourse.mybir as mybir
import concourse.tile as tile
from concourse.bass_utils import run_bass_kernel_spmd

B, H, W = 16, 1024, 1024
N_CORES = 8
IMGS_PER_CORE = B // N_CORES
BAND = 124              # output rows per band
NBANDS = (H + BAND - 1) // BAND   # 9
M = 126                 # matmul output partitions (evens [0:62], odds [64:126])
MM_DT = mybir.dt.float32r

# source per (channel, row-parity, col-parity): conv index 0..3 or "X"
_SEL = {
    (0, 0, 0): "X", (0, 0, 1): 1, (0, 1, 0): 2, (0, 1, 1): 3,   # R
    (1, 0, 0): 0, (1, 0, 1): "X", (1, 1, 0): "X", (1, 1, 1): 0,  # G
    (2, 0, 0): 3, (2, 0, 1): 2, (2, 1, 0): 1, (2, 1, 1): "X",    # B
}


def _build_matrices(k5s):
    """Packed lhsT [128, 18*M]: for each (channel, col-parity) a composite
    (X, S1, S2)-plane triple that applies the selected conv's vertical taps
    (or identity) per output-row parity."""
    packed = np.zeros((128, 18 * M), dtype=np.float32)

    def p_of(r):  # partition of absolute row r within the band tile of r0
        return None

    idx = 0
    for ch in range(3):
        for cp in range(2):
            Ms = [np.zeros((128, M), dtype=np.float32) for _ in range(3)]
            for d in range(BAND):                    # output row r0+d
                m = d // 2 if d % 2 == 0 else 64 + (d - 1) // 2
                src = _SEL[(ch, d % 2, cp)]
                if src == "X":
                    p = (d + 2) // 2 if d % 2 == 0 else 64 + (d + 1) // 2
                    Ms[0][p, m] += 1.0
                    continue
                k5 = k5s[src]
                assert np.allclose(k5[:, 1], k5[:, 3])
                assert np.allclose(k5[:, 0], k5[:, 4])
                for dy in range(-2, 3):
                    r = d + dy
                    p = (r + 2) // 2 if r % 2 == 0 else 64 + (r + 1) // 2
                    Ms[0][p, m] += k5[2 + dy, 2]
                    Ms[1][p, m] += k5[2 + dy, 1]
                    Ms[2][p, m] += k5[2 + dy, 0]
            for pl in range(3):
                packed[:, idx * M:(idx + 1) * M] = Ms[pl]
                idx += 1
    return packed


_CACHE = {}


def _split_waits(nc, max_waits=1):
    """The walrus in this container rejects instructions carrying more than
    one sem wait.  Hoist extra waits onto same-engine NoOps inserted right
    before the offending instruction (sequencer waits are executed in
    program order, so this is semantics-preserving)."""
    total = 0
    for bb in nc.main_func.blocks:
        insts = bb.bb.instructions if hasattr(bb, "bb") else bb.instructions
        i = 0
        while i < len(insts):
            ins = insts[i]
            si = ins.sync_info
            if si is not None and si.on_wait and len(si.on_wait) > max_waits:
                waits = list(si.on_wait)
                keep, hoist = waits[-max_waits:], waits[:-max_waits]
                nops = []
                for w in hoist:
                    nop = mybir.InstNoOp(
                        name=nc.get_next_instruction_name(),
                        engine=ins.engine, ins=[], outs=[],
                        sync_info=mybir.SyncInfo(on_wait=[w], on_update=[]))
                    nc.register_instruction(nop)
                    nops.append(nop)
                ins.sync_info = mybir.SyncInfo(
                    on_wait=keep, on_update=list(si.on_update or []))
                insts[i:i] = nops
                i += len(nops)
                total += len(nops)
            i += 1
    return total


def _build_nc():
    nc = bass.Bass(target_bir_lowering=False, trn_type="TRN2")
    x = nc.dram_tensor("x", [IMGS_PER_CORE, 1, H, W], MM_DT,
                       kind="ExternalInput")
    wts = nc.dram_tensor("wm", [128, 18 * M], MM_DT, kind="ExternalInput")
    zpad = nc.dram_tensor("zpad", [1, W], MM_DT, kind="ExternalInput")
    out = nc.dram_tensor("out", [IMGS_PER_CORE, 3, H, W], mybir.dt.float32,
                         kind="ExternalOutput")

    with tile.TileContext(nc) as tc:
        with (
            tc.tile_pool(name="wpool", bufs=1) as wpool,
            tc.tile_pool(name="xpool", bufs=3) as xpool,
            tc.tile_pool(name="spool", bufs=3) as spool,
            tc.tile_pool(name="opool", bufs=3) as opool,
            tc.tile_pool(name="psum", bufs=1, space="PSUM") as pspool,
        ):
            wt = wpool.tile([128, 18 * M], MM_DT)
            nc.scalar.dma_start(wt[:], wts[:])

            for b in range(IMGS_PER_CORE):
                for t in range(NBANDS):
                    r0 = t * BAND
                    n_rows = min(BAND, H - r0)       # stored rows this band
                    n_me = (n_rows + 1) // 2
                    n_mo = n_rows // 2

                    xt = xpool.tile([128, W + 4], MM_DT, tag="x")
                    # evens r0-2..r0+124 -> [0:64], odds r0-1..r0+125 -> [64:128]
                    for par in range(2):
                        lo, hi = r0 - 2 + par, r0 + BAND + par + 1
                        vlo = lo if lo >= 0 else lo + 2   # keep row parity
                        vhi = min(hi, H)
                        p0 = par * 64 + (vlo - lo) // 2
                        cnt = (vhi - vlo + 1) // 2
                        nc.scalar.dma_start(
                            xt[p0:p0 + cnt, 2:W + 2],
                            x[b, 0, vlo:vhi:2, :])
                        if lo < 0:      # first band: halo rows above image
                            nc.scalar.dma_start(
                                xt[par * 64:par * 64 + 1, 2:W + 2], zpad[:, :])
                        if hi > H:      # last band: first row past the image
                            nc.scalar.dma_start(
                                xt[p0 + cnt:p0 + cnt + 1, 2:W + 2], zpad[:, :])
                    nc.gpsimd.memset(xt[:, 0:2].bitcast(mybir.dt.float32), 0.0)
                    nc.gpsimd.memset(
                        xt[:, W + 2:W + 4].bitcast(mybir.dt.float32), 0.0)

                    s1 = spool.tile([128, W], MM_DT, tag="s1")
                    s2 = spool.tile([128, W], MM_DT, tag="s2")
                    nc.vector.tensor_tensor(
                        s1[:], xt[:, 1:W + 1], xt[:, 3:W + 3], mybir.AluOpType.add)
                    nc.vector.tensor_tensor(
                        s2[:], xt[:, 0:W], xt[:, 4:W + 4], mybir.AluOpType.add)

                    plane = opool.tile([128, 3 * W], mybir.dt.float32, tag="pl")

                    for ci, (ch, cp) in enumerate(
                            (c, p) for c in range(3) for p in range(2)):
                        ps = pspool.tile([M, 512], mybir.dt.float32,
                                         tag=f"ps{ci}", name=f"ps{ci}")
                        rhss = (xt[:, 2 + cp:2 + cp + W:2],
                                s1[:, cp:W:2], s2[:, cp:W:2])
                        for pl in range(3):
                            nc.tensor.matmul(
                                ps[:],
                                wt[:, (ci * 3 + pl) * M:(ci * 3 + pl + 1) * M],
                                rhss[pl],
                                start=(pl == 0), stop=(pl == 2))
                        dst = plane[0:M, ch * W + cp:ch * W + W:2]
                        if ci % 2 == 0:
                            nc.vector.tensor_copy(dst, ps[:])
                        else:
                            nc.scalar.copy(dst, ps[:])

                    # two channel-merged row-parity stores
                    for par, p0, cnt in ((0, 0, n_me), (1, 64, n_mo)):
                        nc.sync.dma_start(
                            out[b, :, r0 + par:r0 + n_rows:2, :].rearrange(
                                "c h w -> h c w"),
                            plane[p0:p0 + cnt, :].rearrange(
                                "p (c w) -> p c w", c=3))

    _split_waits(nc)
    nc.finalize()
    return nc


def _get_nc():
    if "nc" not in _CACHE:
        _CACHE["nc"] = _build_nc()
    return _CACHE["nc"]


def kernel(CFA_inputs, GR_GB, Rg_RB_Bg_BR, Rg_BR_Bg_RB, Rb_BB_Br_RR, _trace=False):
    cfa = np.ascontiguousarray(np.asarray(CFA_inputs, dtype=np.float32))
    k5s = [np.asarray(k, dtype=np.float32)
           for k in (GR_GB, Rg_RB_Bg_BR, Rg_BR_Bg_RB, Rb_BB_Br_RR)]
    nc = _get_nc()

    wm = _build_matrices(k5s)
    zpad = np.zeros((1, W), dtype=np.float32)
    in_maps = [{"x": cfa[c * IMGS_PER_CORE:(c + 1) * IMGS_PER_CORE],
                "wm": wm, "zpad": zpad} for c in range(N_CORES)]

    res = run_bass_kernel_spmd(nc, in_maps, core_ids=list(range(N_CORES)),
                               trace=_trace)
    outs = np.concatenate([res.results[c]["out"] for c in range(N_CORES)], axis=0)
    if _trace:
        kernel._last = res
    return outs
